# revision 1
# baseline (speedup 1.0000x reference)
"""Trainium2 Bass kernel for nn_MultiHeadAttention (N=2048, D=1024, H=16, causal).

Sharding: the 16 heads are split across the 8 NeuronCores (2 heads/core,
tensor-parallel on the head dim, per the sharding hint).  Each core:
  - projects Q^T/K^T (its 128 head-dims x full sequence) and V for its heads,
  - computes causal attention in scores-TRANSPOSED layout ([nk, nq] blocks):
    softmax runs along the nk partition axis with no max-subtraction (scores
    are O(1) here so exp is safe), and the softmax denominator falls out of
    the PV matmul via a ones-column appended to V,
  - applies the 128-column slice of Wo, giving a partial [2048, 1024] output.
The host sums the 8 partial outputs and adds bo (the "all-reduce after W_o"
step; cheaper done host-side than an on-device AllReduce of 8.4 MB/core).

Structure: "t-outer" — for each of the 4 nq column tiles, input column tiles
are DMA-streamed and projected, both heads' attention for that nq range runs
over nk blocks 0..4t+3 (causally trimmed), and normalization + output
projection + output DMA for those 4 row-blocks happen immediately.  This
overlaps input DMA, PE matmuls, ScalarE softmax, and output DMA across the
whole kernel (modeled ~133 us/core vs ~210 us for a phase-serial version).

Dtypes: float32r (TF32-like PE mode, full rate at free-dim >= 256) for all
matmul operands, fp32 PSUM accumulation and softmax.  Measured end-to-end
relative error vs the fp32 jax reference: ~2e-4.
"""
import os
import sys

for _p in ("/opt/trn_rl_repo", "/root/.axon_site/_ro/trn_rl_repo"):
    if os.path.isdir(_p) and _p not in sys.path:
        sys.path.append(_p)

import numpy as np

import concourse.bass as bass
import concourse.mybir as mybir
from concourse import bacc
from concourse.bass_utils import run_bass_kernel_spmd
from concourse.tile import TileContext
from contextlib import ExitStack

N = 2048
D = 1024
NCORES = 8
DL = 128

F32 = mybir.dt.float32
F32R = mybir.dt.float32r
BF16 = mybir.dt.bfloat16

DT = F32R


def build_nc(opts=None):
    o = dict(qk_dt=F32R, v_dt=F32R, pdt=F32R)
    if opts:
        o.update(opts)
    qk_dt = o["qk_dt"]
    v_dt = o["v_dt"]
    pdt = o["pdt"]
    nc = bacc.Bacc("TRN2", target_bir_lowering=False, debug=False,
                   num_devices=NCORES)

    qT = nc.dram_tensor("qT", [D, N], qk_dt, kind="ExternalInput")
    kT = nc.dram_tensor("kT", [D, N], qk_dt, kind="ExternalInput")
    vT = nc.dram_tensor("vT", [D, N], v_dt, kind="ExternalInput")
    wqT = nc.dram_tensor("wqT", [D, DL], qk_dt, kind="ExternalInput")
    wkT = nc.dram_tensor("wkT", [D, DL], qk_dt, kind="ExternalInput")
    wvT = nc.dram_tensor("wvT", [D, DL], v_dt, kind="ExternalInput")
    bqkv = nc.dram_tensor("bqkv", [DL, 3], F32, kind="ExternalInput")
    bvrow = nc.dram_tensor("bvrow", [1, DL], v_dt, kind="ExternalInput")
    woT = nc.dram_tensor("woT", [DL, D], DT, kind="ExternalInput")
    out = nc.dram_tensor("out", [N, D], F32, kind="ExternalOutput")

    AF = mybir.ActivationFunctionType

    with TileContext(nc) as tc, ExitStack() as ctx:
        const = ctx.enter_context(tc.tile_pool(name="const", bufs=1))
        big = ctx.enter_context(tc.tile_pool(name="big", bufs=1))
        stream = ctx.enter_context(tc.tile_pool(name="stream", bufs=12))
        vstream = ctx.enter_context(tc.tile_pool(name="vstream", bufs=10))
        probs_pool = ctx.enter_context(tc.tile_pool(name="probs", bufs=4))
        recip_pool = ctx.enter_context(tc.tile_pool(name="recip", bufs=2))
        outp = ctx.enter_context(tc.tile_pool(name="outp", bufs=4))

        # constants (scalar queue, before any activation work exists)
        wq = const.tile([128, 8, DL], qk_dt)
        nc.scalar.dma_start(wq[:], wqT.rearrange("(j p) d -> p j d", p=128))
        wk = const.tile([128, 8, DL], qk_dt)
        nc.scalar.dma_start(wk[:], wkT.rearrange("(j p) d -> p j d", p=128))
        wv = const.tile([128, 8, DL], v_dt)
        nc.scalar.dma_start(wv[:], wvT.rearrange("(j p) d -> p j d", p=128))
        wo = const.tile([128, D], DT)
        nc.scalar.dma_start(wo[:], woT[:])
        bias_cols = const.tile([128, 3], F32)
        nc.scalar.dma_start(bias_cols[:], bqkv[:])
        if o["v_dt"] != F32R:
            bv_row = const.tile([1, DL], v_dt)
            nc.scalar.dma_start(bv_row[:], bvrow[:])
            ones_n = const.tile([1, 128], v_dt)
            nc.vector.memset(ones_n[:], 1.0)
        ones64 = const.tile([1, 64], F32)
        nc.vector.memset(ones64[:], 1.0)
        if o["v_dt"] == F32R:
            from concourse.masks import make_identity
            ident = const.tile([128, 128], F32)
            make_identity(nc, ident[:])

        QTs = [big.tile([128, 512], DT, name=f"QT{t}") for t in range(4)]
        KTs = [big.tile([128, 512], DT, name=f"KT{t}") for t in range(4)]
        Vaug0 = big.tile([128, 16, 65], pdt)
        Vaug1 = big.tile([128, 16, 65], pdt)
        if pdt == F32R:
            onescol = const.tile([128, 16, 1], F32)
            nc.vector.memset(onescol[:], 1.0)
            nc.vector.tensor_copy(Vaug0[:, :, 64:65], onescol[:])
            nc.vector.tensor_copy(Vaug1[:, :, 64:65], onescol[:])
        else:
            nc.vector.memset(Vaug0[:, :, 64:65], 1.0)
            nc.vector.memset(Vaug1[:, :, 64:65], 1.0)
        attnT_n = big.tile([128, N], DT)
        denom0 = big.tile([1, N], F32)
        denom1 = big.tile([1, N], F32)

        with tc.tile_pool(name="sc_ps", bufs=3, space="PSUM") as sc_ps, \
             tc.tile_pool(name="pv_ps", bufs=1, space="PSUM") as pv_ps, \
             tc.tile_pool(name="proj_ps", bufs=1, space="PSUM") as proj_ps, \
             tc.tile_pool(name="wo_ps", bufs=2, space="PSUM") as wo_ps:

            for t in range(4):
                # ---- Q/K column-tile projections ----
                for src, w, bcol, dst in ((qT, wq, 0, QTs[t]),
                                          (kT, wk, 1, KTs[t])):
                    ps = proj_ps.tile([128, 512], F32, name="proj")
                    for j in range(8):
                        xt = stream.tile([128, 512], qk_dt, name="xc")
                        eng = (nc.sync, nc.scalar)[j % 2]
                        eng.dma_start(
                            xt[:],
                            src[128 * j:128 * (j + 1), 512 * t:512 * (t + 1)])
                        nc.tensor.matmul(ps[:], w[:, j, :], xt[:],
                                         start=(j == 0), stop=(j == 7))
                    nc.vector.tensor_scalar_add(dst[:], ps[:],
                                                bias_cols[:, bcol:bcol + 1])
                # ---- V blocks 4t..4t+3 (layout [n, dk], heads split) ----
                vgc = []
                for j in range(8):
                    vc = vstream.tile([128, 512], v_dt, name="vc")
                    (nc.scalar if j % 2 else nc.sync).dma_start(
                        vc[:], vT[128 * j:128 * (j + 1), 512 * t:512 * (t + 1)])
                    vgc.append(vc)
                if o["v_dt"] == F32R:
                    # VT column tile then PE-transpose into Vaug
                    ps = proj_ps.tile([128, 512], F32, name="proj")
                    for j in range(8):
                        nc.tensor.matmul(ps[:], wv[:, j, :], vgc[j][:],
                                         start=(j == 0), stop=(j == 7))
                    vtt = vstream.tile([128, 512], F32, name="vtt")
                    nc.vector.tensor_scalar_add(vtt[:], ps[:],
                                                bias_cols[:, 2:3])
                    for bb in range(4):
                        b = 4 * t + bb
                        tp = proj_ps.tile([128, 512], F32, name="proj")
                        nc.tensor.transpose(tp[:, 0:128],
                                            vtt[:, 128 * bb:128 * (bb + 1)],
                                            ident[:])
                        nc.vector.tensor_copy(Vaug0[:, b, 0:64], tp[:, 0:64])
                        nc.vector.tensor_copy(Vaug1[:, b, 0:64], tp[:, 64:128])
                else:
                    for bb in range(4):
                        b = 4 * t + bb
                        ps = proj_ps.tile([128, 512], F32, name="proj")
                        for j in range(8):
                            nc.tensor.matmul(ps[:, 0:128],
                                             vgc[j][:, 128 * bb:128 * (bb + 1)],
                                             wv[:, j, :],
                                             start=(j == 0), stop=False)
                        nc.tensor.matmul(ps[:, 0:128], ones_n[:], bv_row[:],
                                         start=False, stop=True)
                        nc.vector.tensor_copy(Vaug0[:, b, 0:64], ps[:, 0:64])
                        nc.vector.tensor_copy(Vaug1[:, b, 0:64], ps[:, 64:128])

                # ---- attention for nq tile t, both heads ----
                for h in range(2):
                    Vaug = (Vaug0, Vaug1)[h]
                    denom = (denom0, denom1)[h]
                    pvh = pv_ps.tile([65, 512], F32, name=f"pvh{h}")
                    prev = None
                    for b in range(4 * t + 4):
                        sc = sc_ps.tile([128, 512], F32, name="sc")
                        nc.tensor.matmul(
                            sc[:],
                            KTs[b // 4][64 * h:64 * (h + 1),
                                        128 * (b % 4):128 * (b % 4 + 1)],
                            QTs[t][64 * h:64 * (h + 1), :],
                            start=True, stop=True, tile_position=(64 * h, 0))
                        probs = probs_pool.tile([128, 512], pdt, name="probs")
                        nc.scalar.activation(probs[:], sc[:], AF.Exp,
                                             scale=0.125)
                        if b >= 4 * t:
                            off = 128 * (b - 4 * t)
                            nc.gpsimd.affine_select(
                                out=probs[:, 0:off + 128],
                                in_=probs[:, 0:off + 128],
                                compare_op=mybir.AluOpType.is_ge, fill=0.0,
                                base=-off, pattern=[[1, off + 128]],
                                channel_multiplier=-1)
                        if prev is not None:
                            pb, pp = prev
                            nc.tensor.matmul(pvh[:], Vaug[:, pb, :], pp[:],
                                             start=(pb == 0),
                                             stop=(pb == 4 * t + 3))
                        prev = (b, probs)
                    pb, pp = prev
                    nc.tensor.matmul(pvh[:], Vaug[:, pb, :], pp[:],
                                     start=(pb == 0), stop=(pb == 4 * t + 3))
                    # finalize softmax for this head / column tile
                    nc.vector.tensor_copy(denom[:, 512 * t:512 * (t + 1)],
                                          pvh[64:65, :])
                    bc = sc_ps.tile([64, 512], F32, name="sc")
                    nc.tensor.matmul(bc[:], ones64[:],
                                     denom[:, 512 * t:512 * (t + 1)],
                                     start=True, stop=True)
                    rc = recip_pool.tile([64, 512], F32, name="rc")
                    nc.vector.reciprocal(rc[:], bc[:])
                    nc.vector.tensor_mul(
                        attnT_n[64 * h:64 * (h + 1), 512 * t:512 * (t + 1)],
                        pvh[0:64, :], rc[:])

                # ---- output projection for row blocks 4t..4t+3 ----
                for m in range(4 * t, 4 * t + 4):
                    for u in range(2):
                        wps = wo_ps.tile([128, 512], F32, name="wo")
                        nc.tensor.matmul(wps[:],
                                         attnT_n[:, 128 * m:128 * (m + 1)],
                                         wo[:, 512 * u:512 * (u + 1)],
                                         start=True, stop=True)
                        ob = outp.tile([128, 512], F32, name="ob")
                        nc.vector.tensor_copy(ob[:], wps[:])
                        oeng = (nc.sync, nc.scalar)[(m + u) % 2] if t == 3 \
                            else nc.sync
                        oeng.dma_start(
                            out[128 * m:128 * (m + 1), 512 * u:512 * (u + 1)],
                            ob[:])

    nc.compile()
    return nc


def make_in_maps(q, k, v, Wq, bq, Wk, bk, Wv, bv, Wo, bo,
                 qk_np=np.float32, v_np=np.float32):
    f32 = np.float32
    qTa = np.ascontiguousarray(q.T).astype(qk_np)
    kTa = np.ascontiguousarray(k.T).astype(qk_np)
    vTa = np.ascontiguousarray(v.T).astype(v_np)
    WqT = np.ascontiguousarray(Wq.T)
    WkT = np.ascontiguousarray(Wk.T)
    WvT = np.ascontiguousarray(Wv.T)
    WoT = np.ascontiguousarray(Wo.T, dtype=f32)
    in_maps = []
    for c in range(NCORES):
        d0 = DL * c
        in_maps.append({
            "qT": qTa, "kT": kTa, "vT": vTa,
            "wqT": np.ascontiguousarray(WqT[:, d0:d0 + DL]).astype(qk_np),
            "wkT": np.ascontiguousarray(WkT[:, d0:d0 + DL]).astype(qk_np),
            "wvT": np.ascontiguousarray(WvT[:, d0:d0 + DL]).astype(v_np),
            "bqkv": np.ascontiguousarray(
                np.stack([bq[d0:d0 + DL], bk[d0:d0 + DL], bv[d0:d0 + DL]],
                         axis=1)).astype(f32),
            "bvrow": bv[d0:d0 + DL].reshape(1, DL).astype(v_np),
            "woT": np.ascontiguousarray(WoT[d0:d0 + DL, :]),
        })
    return in_maps


_NC_CACHE = None


def _get_nc():
    global _NC_CACHE
    if _NC_CACHE is None:
        _NC_CACHE = build_nc()
    return _NC_CACHE


def kernel(q, k, v, Wq, bq, Wk, bk, Wv, bv, Wo, bo):
    """Full-input / full-output entry point (harness contract)."""
    q, k, v = np.asarray(q), np.asarray(k), np.asarray(v)
    Wq, bq, Wk, bk = np.asarray(Wq), np.asarray(bq), np.asarray(Wk), np.asarray(bk)
    Wv, bv, Wo, bo = np.asarray(Wv), np.asarray(bv), np.asarray(Wo), np.asarray(bo)
    nc = _get_nc()
    in_maps = make_in_maps(q, k, v, Wq, bq, Wk, bk, Wv, bv, Wo, bo)
    res = run_bass_kernel_spmd(nc, in_maps, list(range(NCORES)))
    acc = res.results[0]["out"].astype(np.float64)
    for c in range(1, NCORES):
        acc += res.results[c]["out"]
    acc += bo.astype(np.float64)
    return acc.astype(np.float32)



# revision 3
# speedup vs baseline: 1.0041x; 1.0041x over previous
"""Trainium2 Bass kernel for nn_MultiHeadAttention (N=2048, D=1024, H=16, causal).

Sharding: 16 heads split across 8 NeuronCores (2 heads/core, tensor-parallel
per the sharding hint).  Each core projects Q^T/K^T (its 128 head-dims x full
sequence) and V for its heads, computes causal attention in scores-transposed
layout ([nk, nq] blocks, softmax along the nk partition axis with no
max-subtraction; the denominator falls out of a ones-column appended to V),
applies its 128-row slice of Wo, and writes a full [2048, 1024] partial
output.  The host sums the 8 partials and adds bo.

All streamed tensors (q/k/v, weights, partial outputs) are fp16 on the wire
and in the PE: this halves HBM traffic vs fp32 and runs the PE at full rate
at any tile width.  Attention matmuls are causally trimmed at 128-column
granularity (diagonal-block matmuls/exps only cover columns >= the block
offset), and score tiles are paired two-to-a-PSUM-allocation so most exp
activations run 1024 wide.  Input DMAs are 12 large transfers (one per
tensor x 512-column tile); outputs are 16 row-block transfers.
"""
import os
import sys

for _p in ("/opt/trn_rl_repo", "/root/.axon_site/_ro/trn_rl_repo"):
    if os.path.isdir(_p) and _p not in sys.path:
        sys.path.append(_p)

import numpy as np

import concourse.bass as bass
import concourse.mybir as mybir
from concourse import bacc
from concourse.bass_utils import run_bass_kernel_spmd
from concourse.tile import TileContext
from contextlib import ExitStack

N = 2048
D = 1024
NCORES = 8
DL = 128

F32 = mybir.dt.float32
F16 = mybir.dt.float16


def build_nc(opts=None):
    nc = bacc.Bacc("TRN2", target_bir_lowering=False, debug=False,
                   num_devices=NCORES)

    qT = nc.dram_tensor("qT", [D, N], F16, kind="ExternalInput")
    kT = nc.dram_tensor("kT", [D, N], F16, kind="ExternalInput")
    vT = nc.dram_tensor("vT", [D, N], F16, kind="ExternalInput")
    wqkv = nc.dram_tensor("wqkv", [D, 3 * DL], F16, kind="ExternalInput")
    wo_d = nc.dram_tensor("wo", [DL, D], F16, kind="ExternalInput")
    bqk = nc.dram_tensor("bqk", [DL, 3], F32, kind="ExternalInput")
    out = nc.dram_tensor("out", [N, D], F16, kind="ExternalOutput")

    AF = mybir.ActivationFunctionType
    from concourse.masks import make_identity

    with TileContext(nc) as tc, ExitStack() as ctx:
        const = ctx.enter_context(tc.tile_pool(name="const", bufs=1))
        big = ctx.enter_context(tc.tile_pool(name="big", bufs=1))
        stream = ctx.enter_context(tc.tile_pool(name="stream", bufs=1))
        vstage = ctx.enter_context(tc.tile_pool(name="vstage", bufs=2))
        probs_pool = ctx.enter_context(tc.tile_pool(name="probs", bufs=3))
        rc_pool = ctx.enter_context(tc.tile_pool(name="rc", bufs=2))
        outp = ctx.enter_context(tc.tile_pool(name="outp", bufs=3))

        # ---- constants on the scalar queue (before activations exist) ----
        wq = const.tile([128, 8, 3 * DL], F16)
        nc.scalar.dma_start(wq[:], wqkv.rearrange("(j p) c -> p j c", p=128))
        wo = const.tile([128, D], F16)
        nc.scalar.dma_start(wo[:], wo_d[:])
        bias_cols = const.tile([128, 3], F32)
        nc.scalar.dma_start(bias_cols[:], bqk[:])

        ident = const.tile([128, 128], F32)
        make_identity(nc, ident[:])
        ones64 = const.tile([1, 64], F16)
        nc.vector.memset(ones64[:], 1.0)

        # persistent SBUF state
        QTs = [big.tile([128, 512], F16, name=f"QT{t}") for t in range(4)]
        KTs = [big.tile([128, 512], F16, name=f"KT{t}") for t in range(4)]
        Vaug0 = big.tile([128, 16, 65], F16)
        Vaug1 = big.tile([128, 16, 65], F16)
        nc.vector.memset(Vaug0[:, :, 64:65], 1.0)
        nc.vector.memset(Vaug1[:, :, 64:65], 1.0)
        attnT = big.tile([128, N], F16)

        # input stream tiles: one DMA per (tensor, 512-col tile)
        qs = [stream.tile([128, 8, 512], F16, name=f"q{t}") for t in range(4)]
        ks = [stream.tile([128, 8, 512], F16, name=f"k{t}") for t in range(4)]
        vs = [stream.tile([128, 8, 512], F16, name=f"v{t}") for t in range(4)]
        qTr = qT.rearrange("(j p) n -> p j n", p=128)
        kTr = kT.rearrange("(j p) n -> p j n", p=128)
        vTr = vT.rearrange("(j p) n -> p j n", p=128)
        # t=0 inputs on sync (arrive first); the rest on scalar behind consts
        for t in range(4):
            eng = nc.sync if t == 0 else nc.scalar
            eng.dma_start(qs[t][:], qTr[:, :, 512 * t:512 * (t + 1)])
            eng.dma_start(ks[t][:], kTr[:, :, 512 * t:512 * (t + 1)])
            eng.dma_start(vs[t][:], vTr[:, :, 512 * t:512 * (t + 1)])

        with tc.tile_pool(name="sc_ps", bufs=2, space="PSUM") as sc_ps, \
             tc.tile_pool(name="pv_ps", bufs=1, space="PSUM") as pv_ps, \
             tc.tile_pool(name="mm_ps", bufs=3, space="PSUM") as mm_ps:

            for t in range(4):
                # ---- Q/K projections for columns 512t..512t+512 ----
                for c0, bcol, dst in ((0, 0, QTs[t]), (DL, 1, KTs[t])):
                    ps = mm_ps.tile([128, 512], F32, name="mm")
                    for j in range(8):
                        nc.tensor.matmul(ps[:], wq[:, j, c0:c0 + DL],
                                         qs[t][:, j, :] if c0 == 0
                                         else ks[t][:, j, :],
                                         start=(j == 0), stop=(j == 7))
                    nc.vector.tensor_scalar_add(dst[:], ps[:],
                                                bias_cols[:, bcol:bcol + 1])
                # ---- V projection (transposed) + per-block PE transpose ----
                ps = mm_ps.tile([128, 512], F32, name="mm")
                for j in range(8):
                    nc.tensor.matmul(ps[:], wq[:, j, 2 * DL:3 * DL],
                                     vs[t][:, j, :],
                                     start=(j == 0), stop=(j == 7))
                vtt = vstage.tile([128, 512], F32, name="vtt")
                nc.vector.tensor_scalar_add(vtt[:], ps[:], bias_cols[:, 2:3])
                for bb in range(4):
                    b = 4 * t + bb
                    tp = mm_ps.tile([128, 512], F32, name="mm")
                    nc.tensor.transpose(tp[:, 0:128],
                                        vtt[:, 128 * bb:128 * (bb + 1)],
                                        ident[:])
                    nc.vector.tensor_copy(Vaug0[:, b, 0:64], tp[:, 0:64])
                    nc.vector.tensor_copy(Vaug1[:, b, 0:64], tp[:, 64:128])

                # ---- attention for nq tile t, heads h=0,1 ----
                nkb = 4 * t + 4
                for h in range(2):
                    Vaug = (Vaug0, Vaug1)[h]
                    pvh = pv_ps.tile([65, 512], F32, name="pvh")
                    pend = []   # (kb, probs_tile, gi, off) awaiting PV
                    sc = None
                    for kb in range(nkb):
                        g, gi = divmod(kb, 2)
                        off = 128 * (kb - 4 * t) if kb >= 4 * t else 0
                        if gi == 0:
                            sc = sc_ps.tile([128, 2, 512], F32, name="sc")
                            pr = probs_pool.tile([128, 2, 512], F16,
                                                 name="pr")
                        nc.tensor.matmul(
                            sc[:, gi, off:512],
                            KTs[kb // 4][64 * h:64 * (h + 1),
                                         128 * (kb % 4):128 * (kb % 4 + 1)],
                            QTs[t][64 * h:64 * (h + 1), off:512],
                            start=True, stop=True, tile_position=(64 * h, 0))
                        if kb < 4 * t:
                            # full block; exp the pair once complete
                            if gi == 1:
                                nc.scalar.activation(pr[:, :, :], sc[:, :, :],
                                                     AF.Exp, scale=0.125)
                                pend.append((kb - 1, pr, 0, 0))
                                pend.append((kb, pr, 1, 0))
                        else:
                            # diagonal block: trimmed exp + causal mask
                            nc.scalar.activation(pr[:, gi, off:512],
                                                 sc[:, gi, off:512],
                                                 AF.Exp, scale=0.125)
                            nc.gpsimd.affine_select(
                                out=pr[:, gi, off:off + 128],
                                in_=pr[:, gi, off:off + 128],
                                compare_op=mybir.AluOpType.is_ge, fill=0.0,
                                base=0, pattern=[[1, 128]],
                                channel_multiplier=-1)
                            pend.append((kb, pr, gi, off))
                        # drain pending PV one pair behind the score matmuls
                        while len(pend) > 2:
                            pkb, ppr, pgi, poff = pend.pop(0)
                            nc.tensor.matmul(pvh[:, poff:512],
                                             Vaug[:, pkb, :],
                                             ppr[:, pgi, poff:512],
                                             start=(pkb == 0),
                                             stop=(pkb == nkb - 1))
                    for pkb, ppr, pgi, poff in pend:
                        nc.tensor.matmul(pvh[:, poff:512],
                                         Vaug[:, pkb, :],
                                         ppr[:, pgi, poff:512],
                                         start=(pkb == 0),
                                         stop=(pkb == nkb - 1))
                    # softmax normalization: recip of denom row, broadcast
                    # over 64 partitions via PE, multiply into attnT
                    rcr = rc_pool.tile([1, 512], F16, name="rcr")
                    with nc.allow_low_precision(reason="softmax recip row"):
                        nc.vector.reciprocal(rcr[:], pvh[64:65, :])
                    bcp = mm_ps.tile([128, 512], F32, name="mm")
                    nc.tensor.matmul(bcp[0:64, :], ones64[:], rcr[:],
                                     start=True, stop=True)
                    nc.vector.tensor_mul(
                        attnT[64 * h:64 * (h + 1), 512 * t:512 * (t + 1)],
                        pvh[0:64, :], bcp[0:64, :])

                # ---- output projection for row blocks 4t..4t+3 ----
                for m in range(4 * t, 4 * t + 4):
                    ob = outp.tile([128, 1024], F16, name="ob")
                    for u in range(2):
                        wps = mm_ps.tile([128, 512], F32, name="mm")
                        nc.tensor.matmul(wps[:],
                                         attnT[:, 128 * m:128 * (m + 1)],
                                         wo[:, 512 * u:512 * (u + 1)],
                                         start=True, stop=True)
                        nc.vector.tensor_copy(ob[:, 512 * u:512 * (u + 1)],
                                              wps[:])
                    nc.sync.dma_start(out[128 * m:128 * (m + 1), :], ob[:])

    nc.compile()
    return nc


def make_in_maps(q, k, v, Wq, bq, Wk, bk, Wv, bv, Wo, bo):
    f16 = np.float16
    f32 = np.float32
    qTa = np.ascontiguousarray(q.T).astype(f16)
    kTa = np.ascontiguousarray(k.T).astype(f16)
    vTa = np.ascontiguousarray(v.T).astype(f16)
    WqT = Wq.T.astype(f16)
    WkT = Wk.T.astype(f16)
    WvT = Wv.T.astype(f16)
    WoT = Wo.T.astype(f16)
    in_maps = []
    for c in range(NCORES):
        d0 = DL * c
        in_maps.append({
            "qT": qTa, "kT": kTa, "vT": vTa,
            "wqkv": np.ascontiguousarray(
                np.concatenate([WqT[:, d0:d0 + DL], WkT[:, d0:d0 + DL],
                                WvT[:, d0:d0 + DL]], axis=1)),
            "wo": np.ascontiguousarray(WoT[d0:d0 + DL, :]),
            "bqk": np.ascontiguousarray(
                np.stack([bq[d0:d0 + DL], bk[d0:d0 + DL], bv[d0:d0 + DL]],
                         axis=1)).astype(f32),
        })
    return in_maps


_NC_CACHE = None


def _get_nc():
    global _NC_CACHE
    if _NC_CACHE is None:
        _NC_CACHE = build_nc()
    return _NC_CACHE


def kernel(q, k, v, Wq, bq, Wk, bk, Wv, bv, Wo, bo):
    """Full-input / full-output entry point (harness contract)."""
    q, k, v = np.asarray(q), np.asarray(k), np.asarray(v)
    Wq, bq, Wk, bk = np.asarray(Wq), np.asarray(bq), np.asarray(Wk), np.asarray(bk)
    Wv, bv, Wo, bo = np.asarray(Wv), np.asarray(bv), np.asarray(Wo), np.asarray(bo)
    nc = _get_nc()
    in_maps = make_in_maps(q, k, v, Wq, bq, Wk, bk, Wv, bv, Wo, bo)
    res = run_bass_kernel_spmd(nc, in_maps, list(range(NCORES)))
    acc = res.results[0]["out"].astype(np.float64)
    for c in range(1, NCORES):
        acc += res.results[c]["out"]
    acc += bo.astype(np.float64)
    return acc.astype(np.float32)
